# revision 9
# baseline (speedup 1.0000x reference)
"""PoseConsistencyLoss Trainium2 kernel (8-core SPMD Bass/Tile), v3.

Math: the reference's outputs (loss, num_matches, mean_distance) depend only on
the per-landmark min squared distance over all splats:
  - matched = splat_positions[argmin] makes sum(sqerr) == min_dist^2 exactly,
  - so loss = sum(valid*minsq)/max(3*num,1), mean = sum(valid*sqrt(minsq))/max(num,1),
    num = sum(minsq < 1.0).
Sharding: splats split across 8 cores (8192 each); each core computes partial
column-mins of the [8192 x 2048] distance matrix; host gathers the 8 partial
vectors, takes the global min and does the (tiny) masked reduction.

v3 design (477us baseline -> ~160us):
 1. Distance matmul in fp32r (1 cy/col vs fp32's 4): fp32-like accuracy is
    kept by folding hi/lo error-compensation terms into EXTRA CONTRACTION ROWS
    of a single K=21 matmul -- PE time depends only on the moving (splat) dim,
    not K, so the compensation is free. Row pairs (landmark x splat):
      0-2:  hi(c^2)   x 1          3-5:  lo(c^2)  x 1
      6-8:  hi(c)     x (-2s)_hi   9-11: lo(c)    x (-2s)_hi
      12-14:hi(c)     x (-2s)_lo   15-17: 1       x hi(s^2)
      18-20: 1        x lo(s^2)
    (hi = fp32r round-on-write, lo = x - hi; dropped lo*lo term ~2^-24.
     The -2 lives on the splat side so landmark prep needs no extra pass.)
 2. 2-strip PE row tiling (K=21 <= 32): features are duplicated at partition
    bases 0 and 32 and matmuls alternate tile_position (0,0)/(32,0), so two
    MMs stream concurrently -- the PE keeps up even when HAM-throttled cold
    (the bursty duty cycle here never un-throttles it).
 3. Consume: DVE tensor_reduce(min) per [128,2048] PSUM span (the DVE min
    floor on this stack: TTR/ttscan/TS-accum all measured or crash at <= 1
    elem/cy/lane), batched per-mt partials folded by one [128,16,4] reduce.
 4. Setup off the critical path: splat features prepped in a [96,768] packed
    layout (ACT squares || DVE scales), landmark features in [96,128] nat
    layout after a single pose matmul; ScalarE and VectorE work in parallel.
"""

import os
import sys
import time

sys.path.insert(0, "/opt/trn_rl_repo")

import numpy as np

import concourse.bass as bass
import concourse.bacc as bacc
import concourse.tile as tile
from concourse import mybir
from concourse.bass_utils import run_bass_kernel_spmd

# Disk-cache NEFF compiles.
import concourse.bass_utils as _bu
import concourse.bass2jax as _b2j

_orig_compile_bir = _bu.compile_bir_kernel
_NEFF_CACHE = os.environ.get("BASS_NEFF_CACHE_DIR", "/tmp/bass_neff_cache")


def _cached_compile_bir(bir_json, tmpdir, neff_name="file.neff"):
    import hashlib
    import shutil

    h = hashlib.sha256(bir_json).hexdigest()[:24]
    os.makedirs(_NEFF_CACHE, exist_ok=True)
    cpath = os.path.join(_NEFF_CACHE, f"{h}_{neff_name}")
    out = os.path.join(tmpdir, neff_name)
    if os.path.exists(cpath):
        shutil.copyfile(cpath, out)
        return out
    p = _orig_compile_bir(bir_json, tmpdir, neff_name=neff_name)
    try:
        shutil.copyfile(p, cpath)
    except OSError:
        pass
    return p


_bu.compile_bir_kernel = _cached_compile_bir
_b2j.compile_bir_kernel = _cached_compile_bir

F32 = mybir.dt.float32
F32R = mybir.dt.float32r
I32 = mybir.dt.int32
AF = mybir.ActivationFunctionType
ALU = mybir.AluOpType
AX = mybir.AxisListType

BIG = 3.0e38

FULL_CFG = dict(
    n_cores=8,
    s_per_core=8192,  # splats per core
    m_total=2048,     # landmarks
    strips=2,         # PE row-tiling strips (1 or 2)
)

K21 = 21    # contraction rows (9 base + 12 hi/lo compensation)
MMSZ = 512  # matmul moving free dim (one PSUM bank)
SPAN = 2048  # consume span (4 PSUM banks)


def build(cfg):
    C = cfg["n_cores"]
    S = cfg["s_per_core"]
    M = cfg["m_total"]
    NS = cfg.get("strips", 2)
    MT = M // 128
    assert S % SPAN == 0 and SPAN % MMSZ == 0
    NSPAN = S // SPAN           # psum spans per landmark block (4)
    MM_PER_SPAN = SPAN // MMSZ  # matmuls per span (4)

    nc = bacc.Bacc("TRN2", target_bir_lowering=False, debug=False, num_devices=C)

    # ---- I/O ----
    spT_d = nc.dram_tensor("spT", [3, S], F32, kind="ExternalInput")
    lmT_d = nc.dram_tensor("lmT", [3, M], F32, kind="ExternalInput")
    poseT_d = nc.dram_tensor("poseT", [4, 4], F32, kind="ExternalInput")
    konst_d = nc.dram_tensor("konst", [6, S], F32, kind="ExternalInput")  # ones
    part_out_d = nc.dram_tensor("partial", [M], F32, kind="ExternalOutput")

    # round-robin issuing engines for setup DMAs -> parallel DGE queues
    _dmaq = [nc.sync, nc.gpsimd, nc.scalar]
    _dmaqi = [0]

    def dq():
        e = _dmaq[_dmaqi[0] % len(_dmaq)]
        _dmaqi[0] += 1
        return e

    FP = 32 * (NS - 1) + K21  # feature tile partition extent

    with tile.TileContext(nc) as tc:
        with (
            tc.tile_pool(name="persist", bufs=1) as persist,
            tc.tile_pool(name="setup", bufs=1) as setup,
        ):
            # ================= splat features (rhs) =================
            # nat layout: [96, 768]: cols 0:256 = s, 256:512 = s^2, 512:768 = -2s
            natp = 96
            natw = S * 3 // natp  # 256
            nat = setup.tile([natp, 3 * natw], F32)
            nc.sync.dma_start(
                nat[:, 0:natw],
                spT_d[:].rearrange("a b -> (a b)").rearrange("(p f) -> p f", p=natp),
            )
            nc.scalar.activation(nat[:, natw : 2 * natw], nat[:, 0:natw], AF.Square)
            nc.vector.tensor_scalar(
                out=nat[:, 2 * natw : 3 * natw],
                in0=nat[:, 0:natw],
                scalar1=-2.0,
                scalar2=None,
                op0=ALU.mult,
            )
            # hi/lo of [s^2 | -2s] (cols 256:768)
            nat_hi = setup.tile([natp, 2 * natw], F32R)
            nc.vector.tensor_copy(nat_hi[:], nat[:, natw : 3 * natw])
            nat_lo = setup.tile([natp, 2 * natw], F32R)
            nc.vector.tensor_sub(nat_lo[:], nat[:, natw : 3 * natw], nat_hi[:].bitcast(F32))

            # feat_sp rows (per strip at partition base 32*s):
            #   0-5 ones, 6-8 (-2s)_hi, 9-11 (-2s)_hi, 12-14 (-2s)_lo,
            #   15-17 sq_hi, 18-20 sq_lo
            feat_sp = persist.tile([FP, S], F32R)
            for st in range(NS):
                b = 32 * st
                dq().dma_start(feat_sp[b : b + 6, :], konst_d[0:6, :].bitcast(F32R))
                dq().dma_start(feat_sp[b + 6 : b + 9, :], nat_hi[:, natw : 2 * natw])
                dq().dma_start(feat_sp[b + 9 : b + 12, :], nat_hi[:, natw : 2 * natw])
                dq().dma_start(feat_sp[b + 12 : b + 15, :], nat_lo[:, natw : 2 * natw])
                dq().dma_start(feat_sp[b + 15 : b + 18, :], nat_hi[:, 0:natw])
                dq().dma_start(feat_sp[b + 18 : b + 21, :], nat_lo[:, 0:natw])

            # ================= landmark features (lhsT) =================
            pt = setup.tile([4, 4], F32)
            nc.sync.dma_start(pt[:], poseT_d[:])
            homT = setup.tile([4, M], F32)
            nc.sync.dma_start(homT[0:3, :], lmT_d[:])
            nc.sync.dma_start(homT[3:4, :], konst_d[0:1, 0:M])

            # cam = pose @ hom; pk = [c^2 | c] packed [3, 2M]
            pk = setup.tile([3, 2 * M], F32)
            with tc.tile_pool(name="lmpsum", bufs=1, space="PSUM") as lpp:
                cam = lpp.tile([4, M], F32)
                for b in range(M // MMSZ):
                    sl = slice(b * MMSZ, (b + 1) * MMSZ)
                    nc.tensor.matmul(cam[:, sl], pt[:], homT[:, sl], start=True, stop=True)
                nc.scalar.activation(pk[:, 0:M], cam[0:3, :], AF.Square)
                nc.vector.tensor_copy(pk[:, M : 2 * M], cam[0:3, :])
            # nat view [96, 128]: 16-partition blocks: c2x cx c2y cy c2z cz
            pknat = setup.tile([96, 2 * M * 3 // 96], F32)  # [96, 128]
            nc.sync.dma_start(pknat[:], pk[:])
            pknat_hi = setup.tile([96, 128], F32R)
            nc.vector.tensor_copy(pknat_hi[:], pknat[:])
            pknat_lo = setup.tile([96, 128], F32R)
            nc.vector.tensor_sub(pknat_lo[:], pknat[:], pknat_hi[:].bitcast(F32))

            # feat_lm rows (per strip): 0-2 hi(c^2), 3-5 lo(c^2), 6-8 hi(c),
            #   9-11 lo(c), 12-14 hi(c) dup, 15-20 ones
            feat_lm = persist.tile([FP, M], F32R)
            for st in range(NS):
                b = 32 * st
                for d in range(3):  # c^2_d at pknat partitions 32d..32d+16
                    dq().dma_start(feat_lm[b + d : b + d + 1, :], pknat_hi[32 * d : 32 * d + 16, :])
                    dq().dma_start(feat_lm[b + 3 + d : b + 4 + d, :], pknat_lo[32 * d : 32 * d + 16, :])
                    dq().dma_start(feat_lm[b + 6 + d : b + 7 + d, :], pknat_hi[32 * d + 16 : 32 * d + 32, :])
                    dq().dma_start(feat_lm[b + 9 + d : b + 10 + d, :], pknat_lo[32 * d + 16 : 32 * d + 32, :])
                    dq().dma_start(feat_lm[b + 12 + d : b + 13 + d, :], pknat_hi[32 * d + 16 : 32 * d + 32, :])
                dq().dma_start(feat_lm[b + 15 : b + 21, :], konst_d[0:6, 0:M].bitcast(F32R))

            # ================= main loop =================
            pp = tc.alloc_tile_pool(name="psum", bufs=2, space="PSUM")
            minsq = persist.tile([128, MT], F32)
            cols = persist.tile([128, MT * NSPAN], F32)

            def span_mms(ps, mt, si):
                for h in range(MM_PER_SPAN):
                    st = h % NS
                    b = 32 * st
                    off = si * SPAN + h * MMSZ
                    nc.tensor.matmul(
                        ps[:, h * MMSZ : (h + 1) * MMSZ],
                        feat_lm[b : b + K21, mt * 128 : (mt + 1) * 128],
                        feat_sp[b : b + K21, off : off + MMSZ],
                        start=True,
                        stop=True,
                        tile_position=(b, 0),
                    )

            for mt in range(MT):
                for si in range(NSPAN):
                    ps = pp.tile([128, SPAN], F32, tag="ps")
                    span_mms(ps, mt, si)
                    nc.vector.tensor_reduce(
                        cols[:, mt * NSPAN + si : mt * NSPAN + si + 1],
                        ps[:],
                        AX.X,
                        ALU.min,
                    )
            # one batched fold: [128, MT, NSPAN] -> [128, MT]
            nc.vector.tensor_reduce(
                minsq[:],
                cols[:].rearrange("p (mt si) -> p mt si", mt=MT),
                AX.X,
                ALU.min,
            )
            pp.release()

            # per-core partial min out; global min + masked loss on host
            nc.sync.dma_start(
                part_out_d[:].rearrange("(p f) -> p f", p=128), minsq[:]
            )

    nc.compile()
    return nc


def make_in_maps(cfg, splat_positions, camera_pose, landmarks_3d):
    C = cfg["n_cores"]
    S = cfg["s_per_core"]
    sp = np.ascontiguousarray(np.asarray(splat_positions, np.float32))
    pose = np.asarray(camera_pose, np.float32)
    lm = np.asarray(landmarks_3d, np.float32)
    konst = np.ones((6, S), np.float32)
    poseT = np.ascontiguousarray(pose.T)
    lmT = np.ascontiguousarray(lm.T)
    maps = []
    for c in range(C):
        shard = sp[c * S : (c + 1) * S]
        maps.append(
            {
                "spT": np.ascontiguousarray(shard.T),
                "lmT": lmT,
                "poseT": poseT,
                "konst": konst,
            }
        )
    return maps


_COMPILED = None


def _get_compiled():
    global _COMPILED
    if _COMPILED is None:
        _COMPILED = build(FULL_CFG)
    return _COMPILED


def kernel(
    splat_positions,
    camera_pose,
    landmarks_3d,
    landmarks_2d=None,
    camera_intrinsics=None,
    **_unused,
):
    nc = _get_compiled()
    in_maps = make_in_maps(FULL_CFG, splat_positions, camera_pose, landmarks_3d)
    core_ids = list(range(FULL_CFG["n_cores"]))
    try:
        res = run_bass_kernel_spmd(nc, in_maps, core_ids)
    except Exception:
        # one retry -- a previous run can leave the device wedged
        time.sleep(5.0)
        res = run_bass_kernel_spmd(nc, in_maps, core_ids)
    # host-side cross-core min + masked reduction (2048 elements)
    parts = np.stack([r["partial"] for r in res.results], axis=0)
    msq = np.maximum(parts.min(axis=0), np.float32(0.0)).astype(np.float32)
    d = np.sqrt(msq)
    valid = d < np.float32(1.0)
    num = np.int32(valid.sum())
    loss = np.float32(
        (msq * valid).sum(dtype=np.float32)
        / max(np.float32(3.0) * np.float32(num), np.float32(1.0))
    )
    meand = np.float32(
        (d * valid).sum(dtype=np.float32) / max(np.float32(num), np.float32(1.0))
    )
    return loss, num, meand


if __name__ == "__main__":
    build(FULL_CFG)
    print("build ok")


# revision 11
# speedup vs baseline: 1.0471x; 1.0471x over previous
"""PoseConsistencyLoss Trainium2 kernel (8-core SPMD Bass/Tile), v3.

Math: the reference's outputs (loss, num_matches, mean_distance) depend only on
the per-landmark min squared distance over all splats:
  - matched = splat_positions[argmin] makes sum(sqerr) == min_dist^2 exactly,
  - so loss = sum(valid*minsq)/max(3*num,1), mean = sum(valid*sqrt(minsq))/max(num,1),
    num = sum(minsq < 1.0).
Sharding: splats split across 8 cores (8192 each); each core computes partial
column-mins of the [8192 x 2048] distance matrix; host gathers the 8 partial
vectors, takes the global min and does the (tiny) masked reduction.

v3 design (477us baseline -> ~160us):
 1. Distance matmul in fp32r (1 cy/col vs fp32's 4): fp32-like accuracy is
    kept by folding hi/lo error-compensation terms into EXTRA CONTRACTION ROWS
    of a single K=21 matmul -- PE time depends only on the moving (splat) dim,
    not K, so the compensation is free. Row pairs (landmark x splat):
      0-2:  hi(c^2)   x 1          3-5:  lo(c^2)  x 1
      6-8:  hi(c)     x (-2s)_hi   9-11: lo(c)    x (-2s)_hi
      12-14:hi(c)     x (-2s)_lo   15-17: 1       x hi(s^2)
      18-20: 1        x lo(s^2)
    (hi = fp32r round-on-write, lo = x - hi; dropped lo*lo term ~2^-24.
     The -2 lives on the splat side so landmark prep needs no extra pass.)
 2. 2-strip PE row tiling (K=21 <= 32): features are duplicated at partition
    bases 0 and 32 and matmuls alternate tile_position (0,0)/(32,0), so two
    MMs stream concurrently -- the PE keeps up even when HAM-throttled cold
    (the bursty duty cycle here never un-throttles it).
 3. Consume: DVE tensor_reduce(min) per [128,2048] PSUM span (the DVE min
    floor on this stack: TTR/ttscan/TS-accum all measured or crash at <= 1
    elem/cy/lane), batched per-mt partials folded by one [128,16,4] reduce.
 4. Setup off the critical path: splat features prepped in a [96,768] packed
    layout (ACT squares || DVE scales), landmark features in [96,128] nat
    layout after a single pose matmul; ScalarE and VectorE work in parallel.
"""

import os
import sys
import time

sys.path.insert(0, "/opt/trn_rl_repo")

import numpy as np

import concourse.bass as bass
import concourse.bacc as bacc
import concourse.tile as tile
from concourse import mybir
from concourse.bass_utils import run_bass_kernel_spmd

# Disk-cache NEFF compiles.
import concourse.bass_utils as _bu
import concourse.bass2jax as _b2j

_orig_compile_bir = _bu.compile_bir_kernel
_NEFF_CACHE = os.environ.get("BASS_NEFF_CACHE_DIR", "/tmp/bass_neff_cache")


def _cached_compile_bir(bir_json, tmpdir, neff_name="file.neff"):
    import hashlib
    import shutil

    h = hashlib.sha256(bir_json).hexdigest()[:24]
    os.makedirs(_NEFF_CACHE, exist_ok=True)
    cpath = os.path.join(_NEFF_CACHE, f"{h}_{neff_name}")
    out = os.path.join(tmpdir, neff_name)
    if os.path.exists(cpath):
        shutil.copyfile(cpath, out)
        return out
    p = _orig_compile_bir(bir_json, tmpdir, neff_name=neff_name)
    try:
        shutil.copyfile(p, cpath)
    except OSError:
        pass
    return p


_bu.compile_bir_kernel = _cached_compile_bir
_b2j.compile_bir_kernel = _cached_compile_bir

F32 = mybir.dt.float32
F32R = mybir.dt.float32r
I32 = mybir.dt.int32
AF = mybir.ActivationFunctionType
ALU = mybir.AluOpType
AX = mybir.AxisListType

BIG = 3.0e38

FULL_CFG = dict(
    n_cores=8,
    s_per_core=8192,  # splats per core
    m_total=2048,     # landmarks
    strips=1,         # PE row-tiling strips (1 or 2); DVE paces either way
)

K21 = 21    # contraction rows (9 base + 12 hi/lo compensation)
MMSZ = 512  # matmul moving free dim (one PSUM bank)
SPAN = 2048  # consume span (4 PSUM banks)


def build(cfg):
    C = cfg["n_cores"]
    S = cfg["s_per_core"]
    M = cfg["m_total"]
    NS = cfg.get("strips", 2)
    MT = M // 128
    assert S % SPAN == 0 and SPAN % MMSZ == 0
    NSPAN = S // SPAN           # psum spans per landmark block (4)
    MM_PER_SPAN = SPAN // MMSZ  # matmuls per span (4)

    nc = bacc.Bacc("TRN2", target_bir_lowering=False, debug=False, num_devices=C)

    # ---- I/O ----
    spT_d = nc.dram_tensor("spT", [3, S], F32, kind="ExternalInput")
    lmT_d = nc.dram_tensor("lmT", [3, M], F32, kind="ExternalInput")
    poseT_d = nc.dram_tensor("poseT", [4, 4], F32, kind="ExternalInput")
    konst_d = nc.dram_tensor("konst", [6, S], F32, kind="ExternalInput")  # ones
    part_out_d = nc.dram_tensor("partial", [M], F32, kind="ExternalOutput")

    # round-robin issuing engines for setup DMAs -> parallel DGE queues
    _dmaq = [nc.sync, nc.gpsimd, nc.scalar]
    _dmaqi = [0]

    def dq():
        e = _dmaq[_dmaqi[0] % len(_dmaq)]
        _dmaqi[0] += 1
        return e

    FP = 32 * (NS - 1) + K21  # feature tile partition extent

    with tile.TileContext(nc) as tc:
        with (
            tc.tile_pool(name="persist", bufs=1) as persist,
            tc.tile_pool(name="setup", bufs=1) as setup,
        ):
            # ================= splat features (rhs) =================
            # nat layout: [96, 768]: cols 0:256 = s, 256:512 = s^2, 512:768 = -2s
            natp = 96
            natw = S * 3 // natp  # 256
            nat = setup.tile([natp, 3 * natw], F32)
            nc.sync.dma_start(
                nat[:, 0:natw],
                spT_d[:].rearrange("a b -> (a b)").rearrange("(p f) -> p f", p=natp),
            )
            nc.scalar.activation(nat[:, natw : 2 * natw], nat[:, 0:natw], AF.Square)
            nc.vector.tensor_scalar(
                out=nat[:, 2 * natw : 3 * natw],
                in0=nat[:, 0:natw],
                scalar1=-2.0,
                scalar2=None,
                op0=ALU.mult,
            )
            # hi/lo of [s^2 | -2s] (cols 256:768)
            nat_hi = setup.tile([natp, 2 * natw], F32R)
            nc.vector.tensor_copy(nat_hi[:], nat[:, natw : 3 * natw])
            nat_lo = setup.tile([natp, 2 * natw], F32R)
            nc.vector.tensor_sub(nat_lo[:], nat[:, natw : 3 * natw], nat_hi[:].bitcast(F32))

            # feat_sp rows (per strip at partition base 32*s):
            #   0-5 ones, 6-8 (-2s)_hi, 9-11 (-2s)_hi, 12-14 (-2s)_lo,
            #   15-17 sq_hi, 18-20 sq_lo
            feat_sp = persist.tile([FP, S], F32R)
            for st in range(NS):
                b = 32 * st
                dq().dma_start(feat_sp[b : b + 6, :], konst_d[0:6, :].bitcast(F32R))
                dq().dma_start(feat_sp[b + 6 : b + 9, :], nat_hi[:, natw : 2 * natw])
                dq().dma_start(feat_sp[b + 9 : b + 12, :], nat_hi[:, natw : 2 * natw])
                dq().dma_start(feat_sp[b + 12 : b + 15, :], nat_lo[:, natw : 2 * natw])
                dq().dma_start(feat_sp[b + 15 : b + 18, :], nat_hi[:, 0:natw])
                dq().dma_start(feat_sp[b + 18 : b + 21, :], nat_lo[:, 0:natw])

            # ================= landmark features (lhsT) =================
            pt = setup.tile([4, 4], F32)
            nc.sync.dma_start(pt[:], poseT_d[:])
            homT = setup.tile([4, M], F32)
            nc.sync.dma_start(homT[0:3, :], lmT_d[:])
            nc.sync.dma_start(homT[3:4, :], konst_d[0:1, 0:M])

            # cam = pose @ hom; pk = [c^2 | c] packed [3, 2M]
            pk = setup.tile([3, 2 * M], F32)
            with tc.tile_pool(name="lmpsum", bufs=1, space="PSUM") as lpp:
                cam = lpp.tile([4, M], F32)
                for b in range(M // MMSZ):
                    sl = slice(b * MMSZ, (b + 1) * MMSZ)
                    nc.tensor.matmul(cam[:, sl], pt[:], homT[:, sl], start=True, stop=True)
                nc.scalar.activation(pk[:, 0:M], cam[0:3, :], AF.Square)
                nc.vector.tensor_copy(pk[:, M : 2 * M], cam[0:3, :])
            # nat view [96, 128]: partitions 0-47 hold c^2 (x,y,z), 48-95 hold c
            pknat = setup.tile([96, 2 * M * 3 // 96], F32)  # [96, 128]
            nc.sync.dma_start(pknat[0:48, :], pk[:, 0:M])
            nc.gpsimd.dma_start(pknat[48:96, :], pk[:, M : 2 * M])
            pknat_hi = setup.tile([96, 128], F32R)
            nc.vector.tensor_copy(pknat_hi[:], pknat[:])
            pknat_lo = setup.tile([96, 128], F32R)
            nc.vector.tensor_sub(pknat_lo[:], pknat[:], pknat_hi[:].bitcast(F32))

            # feat_lm rows (per strip): 0-2 hi(c^2), 3-5 lo(c^2), 6-8 hi(c),
            #   9-11 lo(c), 12-14 hi(c) dup, 15-20 ones
            feat_lm = persist.tile([FP, M], F32R)
            for st in range(NS):
                b = 32 * st
                dq().dma_start(feat_lm[b + 0 : b + 3, :], pknat_hi[0:48, :])
                dq().dma_start(feat_lm[b + 3 : b + 6, :], pknat_lo[0:48, :])
                dq().dma_start(feat_lm[b + 6 : b + 9, :], pknat_hi[48:96, :])
                dq().dma_start(feat_lm[b + 9 : b + 12, :], pknat_lo[48:96, :])
                dq().dma_start(feat_lm[b + 12 : b + 15, :], pknat_hi[48:96, :])
                dq().dma_start(feat_lm[b + 15 : b + 21, :], konst_d[0:6, 0:M].bitcast(F32R))

            # ================= main loop =================
            pp = tc.alloc_tile_pool(name="psum", bufs=2, space="PSUM")
            minsq = persist.tile([128, MT], F32)
            cols = persist.tile([128, MT * NSPAN], F32)

            def span_mms(ps, mt, si):
                for h in range(MM_PER_SPAN):
                    st = h % NS
                    b = 32 * st
                    off = si * SPAN + h * MMSZ
                    nc.tensor.matmul(
                        ps[:, h * MMSZ : (h + 1) * MMSZ],
                        feat_lm[b : b + K21, mt * 128 : (mt + 1) * 128],
                        feat_sp[b : b + K21, off : off + MMSZ],
                        start=True,
                        stop=True,
                        tile_position=(b, 0),
                    )

            for mt in range(MT):
                for si in range(NSPAN):
                    ps = pp.tile([128, SPAN], F32, tag="ps")
                    span_mms(ps, mt, si)
                    nc.vector.tensor_reduce(
                        cols[:, mt * NSPAN + si : mt * NSPAN + si + 1],
                        ps[:],
                        AX.X,
                        ALU.min,
                    )
            # one batched fold: [128, MT, NSPAN] -> [128, MT]
            nc.vector.tensor_reduce(
                minsq[:],
                cols[:].rearrange("p (mt si) -> p mt si", mt=MT),
                AX.X,
                ALU.min,
            )
            pp.release()

            # per-core partial min out; global min + masked loss on host
            nc.sync.dma_start(
                part_out_d[:].rearrange("(p f) -> p f", p=128), minsq[:]
            )

    nc.compile()
    return nc


def make_in_maps(cfg, splat_positions, camera_pose, landmarks_3d):
    C = cfg["n_cores"]
    S = cfg["s_per_core"]
    sp = np.ascontiguousarray(np.asarray(splat_positions, np.float32))
    pose = np.asarray(camera_pose, np.float32)
    lm = np.asarray(landmarks_3d, np.float32)
    konst = np.ones((6, S), np.float32)
    poseT = np.ascontiguousarray(pose.T)
    lmT = np.ascontiguousarray(lm.T)
    maps = []
    for c in range(C):
        shard = sp[c * S : (c + 1) * S]
        maps.append(
            {
                "spT": np.ascontiguousarray(shard.T),
                "lmT": lmT,
                "poseT": poseT,
                "konst": konst,
            }
        )
    return maps


_COMPILED = None


def _get_compiled():
    global _COMPILED
    if _COMPILED is None:
        _COMPILED = build(FULL_CFG)
    return _COMPILED


def kernel(
    splat_positions,
    camera_pose,
    landmarks_3d,
    landmarks_2d=None,
    camera_intrinsics=None,
    **_unused,
):
    nc = _get_compiled()
    in_maps = make_in_maps(FULL_CFG, splat_positions, camera_pose, landmarks_3d)
    core_ids = list(range(FULL_CFG["n_cores"]))
    try:
        res = run_bass_kernel_spmd(nc, in_maps, core_ids)
    except Exception:
        # one retry -- a previous run can leave the device wedged
        time.sleep(5.0)
        res = run_bass_kernel_spmd(nc, in_maps, core_ids)
    # host-side cross-core min + masked reduction (2048 elements)
    parts = np.stack([r["partial"] for r in res.results], axis=0)
    msq = np.maximum(parts.min(axis=0), np.float32(0.0)).astype(np.float32)
    d = np.sqrt(msq)
    valid = d < np.float32(1.0)
    num = np.int32(valid.sum())
    loss = np.float32(
        (msq * valid).sum(dtype=np.float32)
        / max(np.float32(3.0) * np.float32(num), np.float32(1.0))
    )
    meand = np.float32(
        (d * valid).sum(dtype=np.float32) / max(np.float32(num), np.float32(1.0))
    )
    return loss, num, meand


if __name__ == "__main__":
    build(FULL_CFG)
    print("build ok")


# revision 12
# speedup vs baseline: 1.1165x; 1.0662x over previous
"""PoseConsistencyLoss Trainium2 kernel (8-core SPMD Bass/Tile), v3.

Math: the reference's outputs (loss, num_matches, mean_distance) depend only on
the per-landmark min squared distance over all splats:
  - matched = splat_positions[argmin] makes sum(sqerr) == min_dist^2 exactly,
  - so loss = sum(valid*minsq)/max(3*num,1), mean = sum(valid*sqrt(minsq))/max(num,1),
    num = sum(minsq < 1.0).
Sharding: splats split across 8 cores (8192 each); each core computes partial
column-mins of the [8192 x 2048] distance matrix; host gathers the 8 partial
vectors, takes the global min and does the (tiny) masked reduction.

v3 design (477us baseline -> ~160us):
 1. Distance matmul in fp32r (1 cy/col vs fp32's 4): fp32-like accuracy is
    kept by folding hi/lo error-compensation terms into EXTRA CONTRACTION ROWS
    of a single K=21 matmul -- PE time depends only on the moving (splat) dim,
    not K, so the compensation is free. Row pairs (landmark x splat):
      0-2:  hi(c^2)   x 1          3-5:  lo(c^2)  x 1
      6-8:  hi(c)     x (-2s)_hi   9-11: lo(c)    x (-2s)_hi
      12-14:hi(c)     x (-2s)_lo   15-17: 1       x hi(s^2)
      18-20: 1        x lo(s^2)
    (hi = fp32r round-on-write, lo = x - hi; dropped lo*lo term ~2^-24.
     The -2 lives on the splat side so landmark prep needs no extra pass.)
 2. 2-strip PE row tiling (K=21 <= 32): features are duplicated at partition
    bases 0 and 32 and matmuls alternate tile_position (0,0)/(32,0), so two
    MMs stream concurrently -- the PE keeps up even when HAM-throttled cold
    (the bursty duty cycle here never un-throttles it).
 3. Consume: DVE tensor_reduce(min) per [128,2048] PSUM span (the DVE min
    floor on this stack: TTR/ttscan/TS-accum all measured or crash at <= 1
    elem/cy/lane), batched per-mt partials folded by one [128,16,4] reduce.
 4. Setup off the critical path: splat features prepped in a [96,768] packed
    layout (ACT squares || DVE scales), landmark features in [96,128] nat
    layout after a single pose matmul; ScalarE and VectorE work in parallel.
"""

import os
import sys
import time

sys.path.insert(0, "/opt/trn_rl_repo")

import numpy as np

import concourse.bass as bass
import concourse.bacc as bacc
import concourse.tile as tile
from concourse import mybir
from concourse.bass_utils import run_bass_kernel_spmd

# Disk-cache NEFF compiles.
import concourse.bass_utils as _bu
import concourse.bass2jax as _b2j

_orig_compile_bir = _bu.compile_bir_kernel
_NEFF_CACHE = os.environ.get("BASS_NEFF_CACHE_DIR", "/tmp/bass_neff_cache")


def _cached_compile_bir(bir_json, tmpdir, neff_name="file.neff"):
    import hashlib
    import shutil

    h = hashlib.sha256(bir_json).hexdigest()[:24]
    os.makedirs(_NEFF_CACHE, exist_ok=True)
    cpath = os.path.join(_NEFF_CACHE, f"{h}_{neff_name}")
    out = os.path.join(tmpdir, neff_name)
    if os.path.exists(cpath):
        shutil.copyfile(cpath, out)
        return out
    p = _orig_compile_bir(bir_json, tmpdir, neff_name=neff_name)
    try:
        shutil.copyfile(p, cpath)
    except OSError:
        pass
    return p


_bu.compile_bir_kernel = _cached_compile_bir
_b2j.compile_bir_kernel = _cached_compile_bir

F32 = mybir.dt.float32
F32R = mybir.dt.float32r
I32 = mybir.dt.int32
AF = mybir.ActivationFunctionType
ALU = mybir.AluOpType
AX = mybir.AxisListType

BIG = 3.0e38

FULL_CFG = dict(
    n_cores=8,
    s_per_core=8192,  # splats per core
    m_total=2048,     # landmarks
    strips=1,         # PE row-tiling strips (1 or 2); DVE paces either way
)

K21 = 21    # contraction rows (9 base + 12 hi/lo compensation)
MMSZ = 512  # matmul moving free dim (one PSUM bank)
SPAN = 2048  # consume span (4 PSUM banks)


def build(cfg):
    C = cfg["n_cores"]
    S = cfg["s_per_core"]
    M = cfg["m_total"]
    NS = cfg.get("strips", 2)
    MT = M // 128
    assert S % SPAN == 0 and SPAN % MMSZ == 0
    NSPAN = S // SPAN           # psum spans per landmark block (4)
    MM_PER_SPAN = SPAN // MMSZ  # matmuls per span (4)

    nc = bacc.Bacc("TRN2", target_bir_lowering=False, debug=False, num_devices=C)

    # ---- I/O ----
    spT_d = nc.dram_tensor("spT", [3, S], F32, kind="ExternalInput")
    lmT_d = nc.dram_tensor("lmT", [3, M], F32, kind="ExternalInput")
    poseT_d = nc.dram_tensor("poseT", [4, 4], F32, kind="ExternalInput")
    konst_d = nc.dram_tensor("konst", [6, S], F32, kind="ExternalInput")  # ones
    part_out_d = nc.dram_tensor("partial", [M], F32, kind="ExternalOutput")

    # round-robin issuing engines for setup DMAs -> parallel DGE queues
    _dmaq = [nc.sync, nc.gpsimd, nc.scalar]
    _dmaqi = [0]

    def dq():
        e = _dmaq[_dmaqi[0] % len(_dmaq)]
        _dmaqi[0] += 1
        return e

    FP = 32 * (NS - 1) + K21  # feature tile partition extent

    with tile.TileContext(nc) as tc:
        with (
            tc.tile_pool(name="persist", bufs=1) as persist,
            tc.tile_pool(name="setup", bufs=1) as setup,
        ):
            # ================= landmark features (lhsT) =================
            pt = setup.tile([4, 4], F32)
            nc.sync.dma_start(pt[:], poseT_d[:])
            homT = setup.tile([4, M], F32)
            nc.sync.dma_start(homT[0:3, :], lmT_d[:])
            nc.sync.dma_start(homT[3:4, :], konst_d[0:1, 0:M])

            # cam = pose @ hom; pk = [c^2 | c] packed [3, 2M]
            pk = setup.tile([3, 2 * M], F32)
            with tc.tile_pool(name="lmpsum", bufs=1, space="PSUM") as lpp:
                cam = lpp.tile([4, M], F32)
                for b in range(M // MMSZ):
                    sl = slice(b * MMSZ, (b + 1) * MMSZ)
                    nc.tensor.matmul(cam[:, sl], pt[:], homT[:, sl], start=True, stop=True)
                nc.scalar.activation(pk[:, 0:M], cam[0:3, :], AF.Square)
                nc.vector.tensor_copy(pk[:, M : 2 * M], cam[0:3, :])
            # nat view [96, 128]: partitions 0-47 hold c^2 (x,y,z), 48-95 hold c
            pknat = setup.tile([96, 2 * M * 3 // 96], F32)  # [96, 128]
            nc.sync.dma_start(pknat[0:48, :], pk[:, 0:M])
            nc.gpsimd.dma_start(pknat[48:96, :], pk[:, M : 2 * M])
            pknat_hi = setup.tile([96, 128], F32R)
            nc.vector.tensor_copy(pknat_hi[:], pknat[:])
            pknat_lo = setup.tile([96, 128], F32R)
            nc.vector.tensor_sub(pknat_lo[:], pknat[:], pknat_hi[:].bitcast(F32))

            # feat_lm rows (per strip): 0-2 hi(c^2), 3-5 lo(c^2), 6-8 hi(c),
            #   9-11 lo(c), 12-14 hi(c) dup, 15-20 ones
            feat_lm = persist.tile([FP, M], F32R)
            for st in range(NS):
                b = 32 * st
                dq().dma_start(feat_lm[b + 0 : b + 3, :], pknat_hi[0:48, :])
                dq().dma_start(feat_lm[b + 3 : b + 6, :], pknat_lo[0:48, :])
                dq().dma_start(feat_lm[b + 6 : b + 9, :], pknat_hi[48:96, :])
                dq().dma_start(feat_lm[b + 9 : b + 12, :], pknat_lo[48:96, :])
                dq().dma_start(feat_lm[b + 12 : b + 15, :], pknat_hi[48:96, :])
                dq().dma_start(feat_lm[b + 15 : b + 21, :], konst_d[0:6, 0:M].bitcast(F32R))

            # ================= splat features (rhs) =================
            # nat layout: [96, 768]: cols 0:256 = s, 256:512 = s^2, 512:768 = -2s
            natp = 96
            natw = S * 3 // natp  # 256
            nat = setup.tile([natp, 3 * natw], F32)
            nc.gpsimd.dma_start(
                nat[:, 0:natw],
                spT_d[:].rearrange("a b -> (a b)").rearrange("(p f) -> p f", p=natp),
            )
            nc.scalar.activation(nat[:, natw : 2 * natw], nat[:, 0:natw], AF.Square)
            nc.vector.tensor_scalar(
                out=nat[:, 2 * natw : 3 * natw],
                in0=nat[:, 0:natw],
                scalar1=-2.0,
                scalar2=None,
                op0=ALU.mult,
            )
            # hi/lo of [s^2 | -2s] (cols 256:768)
            nat_hi = setup.tile([natp, 2 * natw], F32R)
            nc.vector.tensor_copy(nat_hi[:], nat[:, natw : 3 * natw])
            nat_lo = setup.tile([natp, 2 * natw], F32R)
            nc.vector.tensor_sub(nat_lo[:], nat[:, natw : 3 * natw], nat_hi[:].bitcast(F32))

            # feat_sp rows (per strip at partition base 32*s):
            #   0-5 ones, 6-8 (-2s)_hi, 9-11 (-2s)_hi, 12-14 (-2s)_lo,
            #   15-17 sq_hi, 18-20 sq_lo
            feat_sp = persist.tile([FP, S], F32R)
            for st in range(NS):
                b = 32 * st
                dq().dma_start(feat_sp[b : b + 6, :], konst_d[0:6, :].bitcast(F32R))
                dq().dma_start(feat_sp[b + 6 : b + 9, :], nat_hi[:, natw : 2 * natw])
                dq().dma_start(feat_sp[b + 9 : b + 12, :], nat_hi[:, natw : 2 * natw])
                dq().dma_start(feat_sp[b + 12 : b + 15, :], nat_lo[:, natw : 2 * natw])
                dq().dma_start(feat_sp[b + 15 : b + 18, :], nat_hi[:, 0:natw])
                dq().dma_start(feat_sp[b + 18 : b + 21, :], nat_lo[:, 0:natw])

            # ================= main loop =================
            pp = tc.alloc_tile_pool(name="psum", bufs=2, space="PSUM")
            minsq = persist.tile([128, MT], F32)
            cols = persist.tile([128, MT * NSPAN], F32)

            def span_mms(ps, mt, si):
                for h in range(MM_PER_SPAN):
                    st = h % NS
                    b = 32 * st
                    off = si * SPAN + h * MMSZ
                    nc.tensor.matmul(
                        ps[:, h * MMSZ : (h + 1) * MMSZ],
                        feat_lm[b : b + K21, mt * 128 : (mt + 1) * 128],
                        feat_sp[b : b + K21, off : off + MMSZ],
                        start=True,
                        stop=True,
                        tile_position=(b, 0),
                    )

            for mt in range(MT):
                for si in range(NSPAN):
                    ps = pp.tile([128, SPAN], F32, tag="ps")
                    span_mms(ps, mt, si)
                    nc.vector.tensor_reduce(
                        cols[:, mt * NSPAN + si : mt * NSPAN + si + 1],
                        ps[:],
                        AX.X,
                        ALU.min,
                    )
            # one batched fold: [128, MT, NSPAN] -> [128, MT]
            nc.vector.tensor_reduce(
                minsq[:],
                cols[:].rearrange("p (mt si) -> p mt si", mt=MT),
                AX.X,
                ALU.min,
            )
            pp.release()

            # per-core partial min out; global min + masked loss on host
            nc.sync.dma_start(
                part_out_d[:].rearrange("(p f) -> p f", p=128), minsq[:]
            )

    nc.compile()
    return nc


def make_in_maps(cfg, splat_positions, camera_pose, landmarks_3d):
    C = cfg["n_cores"]
    S = cfg["s_per_core"]
    sp = np.ascontiguousarray(np.asarray(splat_positions, np.float32))
    pose = np.asarray(camera_pose, np.float32)
    lm = np.asarray(landmarks_3d, np.float32)
    konst = np.ones((6, S), np.float32)
    poseT = np.ascontiguousarray(pose.T)
    lmT = np.ascontiguousarray(lm.T)
    maps = []
    for c in range(C):
        shard = sp[c * S : (c + 1) * S]
        maps.append(
            {
                "spT": np.ascontiguousarray(shard.T),
                "lmT": lmT,
                "poseT": poseT,
                "konst": konst,
            }
        )
    return maps


_COMPILED = None


def _get_compiled():
    global _COMPILED
    if _COMPILED is None:
        _COMPILED = build(FULL_CFG)
    return _COMPILED


def kernel(
    splat_positions,
    camera_pose,
    landmarks_3d,
    landmarks_2d=None,
    camera_intrinsics=None,
    **_unused,
):
    nc = _get_compiled()
    in_maps = make_in_maps(FULL_CFG, splat_positions, camera_pose, landmarks_3d)
    core_ids = list(range(FULL_CFG["n_cores"]))
    try:
        res = run_bass_kernel_spmd(nc, in_maps, core_ids)
    except Exception:
        # one retry -- a previous run can leave the device wedged
        time.sleep(5.0)
        res = run_bass_kernel_spmd(nc, in_maps, core_ids)
    # host-side cross-core min + masked reduction (2048 elements)
    parts = np.stack([r["partial"] for r in res.results], axis=0)
    msq = np.maximum(parts.min(axis=0), np.float32(0.0)).astype(np.float32)
    d = np.sqrt(msq)
    valid = d < np.float32(1.0)
    num = np.int32(valid.sum())
    loss = np.float32(
        (msq * valid).sum(dtype=np.float32)
        / max(np.float32(3.0) * np.float32(num), np.float32(1.0))
    )
    meand = np.float32(
        (d * valid).sum(dtype=np.float32) / max(np.float32(num), np.float32(1.0))
    )
    return loss, num, meand


if __name__ == "__main__":
    build(FULL_CFG)
    print("build ok")
